# revision 1
# baseline (speedup 1.0000x reference)
"""GroupViT cross-attention layer on 8 TRN2 NeuronCores.

Strategy: pure data-parallel over batch (16 batches -> 2 per core, zero
collectives). Feature-major ("transposed") layout on chip: activations
stored [feature(partition), token(free)], weights host-transposed to
[d_in, d_out] so every matmul contracts over the partition dim.

dtypes: attention path bf16 (its output is ~1% of the residual stream,
errors diluted ~86x), MLP/residual/LN path float32r (~1e-4 matmul error
at full PE speed for free-dim >= 256).

Softmax: scores are O(+-3) so exp needs no max subtraction. Scores are
computed transposed [s, t]; denominators come free from a ones column
appended to V in the ctx matmul; normalization via a k=1 outer-product
broadcast matmul + one DVE multiply per head.

LN over the partition (feature) dim: sums via ones-column matmuls,
(x - mu)*rs*g + b applied as x (*) (g (x) rs) - (g (x) mu*rs - b (x) 1)
with the broadcast tensors built by tiny k=1 matmuls into PSUM.
"""

import numpy as np

B, T, S, D, H, HD, FF = 16, 512, 2048, 768, 12, 64, 3072
NCORES = 8
BPC = B // NCORES      # batches per core
P = 128
DC = D // P            # 6 feature chunks
SC = S // P            # 16 key-token chunks
FFC = FF // P          # 24
EPS = 1e-5
SCALE = HD ** -0.5

_cached = {}


def _build(use_bv: bool):
    import concourse.bacc as bacc
    import concourse.tile as tile
    import concourse.mybir as mybir

    f32 = mybir.dt.float32
    f32r = mybir.dt.float32r
    bf16 = mybir.dt.bfloat16
    AF = mybir.ActivationFunctionType
    ALU = mybir.AluOpType

    nc = bacc.Bacc("TRN2", target_bir_lowering=False, debug=False,
                   num_devices=NCORES)

    # ---- DRAM I/O (per-core shapes) ----
    qT_d = nc.dram_tensor("qT", [BPC, D, T], f32r, kind="ExternalInput")
    kT_d = nc.dram_tensor("kT", [BPC, D, S], f32, kind="ExternalInput")
    wq_d = nc.dram_tensor("wq_t", [D, D], f32r, kind="ExternalInput")
    wk_d = nc.dram_tensor("wk_t", [D, D], f32, kind="ExternalInput")
    wv_d = nc.dram_tensor("wv_t", [D, D], f32, kind="ExternalInput")
    wo_d = nc.dram_tensor("wo_t", [D, D], f32, kind="ExternalInput")
    fc1_d = nc.dram_tensor("fc1_t", [D, FF], f32r, kind="ExternalInput")
    fc2_d = nc.dram_tensor("fc2_t", [FF, D], f32r, kind="ExternalInput")
    bq_d = nc.dram_tensor("bqv", [D], f32, kind="ExternalInput")
    bk_d = nc.dram_tensor("bkv", [D], f32, kind="ExternalInput")
    bv_d = nc.dram_tensor("bvv", [1, D], f32r, kind="ExternalInput")
    bo_d = nc.dram_tensor("bov", [D], f32, kind="ExternalInput")
    f1b_d = nc.dram_tensor("f1b", [FF], f32, kind="ExternalInput")
    f2b_d = nc.dram_tensor("f2b", [D], f32, kind="ExternalInput")
    ln2g_d = nc.dram_tensor("ln2g", [1, D], f32r, kind="ExternalInput")
    ln2bn_d = nc.dram_tensor("ln2bn", [1, D], f32r, kind="ExternalInput")
    lnpg_d = nc.dram_tensor("lnpg", [1, D], f32r, kind="ExternalInput")
    lnpbn_d = nc.dram_tensor("lnpbn", [1, D], f32r, kind="ExternalInput")
    ones_col_d = nc.dram_tensor("ones_col", [P, 1], f32r, kind="ExternalInput")
    ones_row_d = nc.dram_tensor("ones_row", [1, T], f32r, kind="ExternalInput")
    out_d = nc.dram_tensor("out", [BPC, D, T], f32, kind="ExternalOutput")

    def F(ap):
        return ap.bitcast(f32)

    def act_reciprocal(out, in_):
        eng = nc.scalar
        ins = [eng.lower_ap(in_)]
        for v in (0.0, 1.0, 0.0):
            ins.append(mybir.ImmediateValue(dtype=f32, value=v))
        return eng.add_instruction(mybir.InstActivation(
            name=nc.get_next_instruction_name(),
            func=AF.Reciprocal, ins=ins, outs=[eng.lower_ap(out)]))

    with tile.TileContext(nc) as tc:
        with (
            tc.tile_pool(name="act", bufs=3) as act,
            tc.tile_pool(name="bigk", bufs=1) as bigk,
            tc.tile_pool(name="vpool", bufs=1) as vpool,
            tc.tile_pool(name="qtp", bufs=1) as qtp,
            tc.tile_pool(name="ktc", bufs=2) as ktc,
            tc.tile_pool(name="wstream", bufs=2) as wstream,
            tc.tile_pool(name="wvp", bufs=1) as wvp,
            tc.tile_pool(name="fstream", bufs=2) as fstream,
            tc.tile_pool(name="expp", bufs=3) as expp,
            tc.tile_pool(name="mchunk", bufs=3) as mchunkp,
            tc.tile_pool(name="tmp", bufs=3) as tmpp,
            tc.tile_pool(name="small", bufs=1) as small,
        ):
            # ---- persistent small tiles ----
            ones_col = small.tile([P, 1], f32r, tag="ones_col")
            nc.sync.dma_start(ones_col[:], ones_col_d.ap())
            ones_row = small.tile([1, T], f32r, tag="ones_row")
            nc.sync.dma_start(ones_row[:], ones_row_d.ap())
            ones64_f = small.tile([1, HD], f32, tag="ones64f")
            nc.vector.memset(ones64_f[:], 1.0)
            ones64_bf = small.tile([1, HD], bf16, tag="ones64")
            nc.vector.tensor_copy(ones64_bf[:], ones64_f[:])
            onesc_f = small.tile([P, 1], f32, tag="onesc_f")
            nc.vector.memset(onesc_f[:], 1.0)
            eps_t = small.tile([1, 1], f32, tag="eps")
            nc.vector.memset(eps_t[:], EPS)

            ln2g = small.tile([1, D], f32r, tag="ln2g")
            nc.sync.dma_start(ln2g[:], ln2g_d.ap())
            ln2bn = small.tile([1, D], f32r, tag="ln2bn")
            nc.sync.dma_start(ln2bn[:], ln2bn_d.ap())
            lnpg = small.tile([1, D], f32r, tag="lnpg")
            nc.sync.dma_start(lnpg[:], lnpg_d.ap())
            lnpbn = small.tile([1, D], f32r, tag="lnpbn")
            nc.sync.dma_start(lnpbn[:], lnpbn_d.ap())

            bq_pc = small.tile([P, DC], f32, tag="bq_pc")
            nc.sync.dma_start(bq_pc[:], bq_d.ap().rearrange("(c p) -> p c", p=P))
            bk_pc = small.tile([P, DC], f32, tag="bk_pc")
            nc.sync.dma_start(bk_pc[:], bk_d.ap().rearrange("(c p) -> p c", p=P))
            bo_pc = small.tile([P, DC], f32, tag="bo_pc")
            nc.sync.dma_start(bo_pc[:], bo_d.ap().rearrange("(c p) -> p c", p=P))
            f1b_pc = small.tile([P, FFC], f32, tag="f1b_pc")
            nc.sync.dma_start(f1b_pc[:], f1b_d.ap().rearrange("(c p) -> p c", p=P))
            f2b_pc = small.tile([P, DC], f32, tag="f2b_pc")
            nc.sync.dma_start(f2b_pc[:], f2b_d.ap().rearrange("(c p) -> p c", p=P))

            bv_row = None
            if use_bv:
                bv_row = small.tile([1, D], f32r, tag="bv_row")
                nc.sync.dma_start(bv_row[:], bv_d.ap())

            def ln_pass(xsrc, dst, g_row, bn_row, ps_scope):
                """LayerNorm over the partition(feature) dim:
                xsrc [P, DC, T] f32r -> dst [P, DC, T]."""
                ps_st, ps_bc = ps_scope
                psum_mu = ps_st.tile([1, T], f32, tag="st_mu")
                psum_sq = ps_st.tile([1, T], f32, tag="st_sq")
                for c in range(DC):
                    nc.tensor.matmul(psum_mu[:], ones_col[:], xsrc[:, c, :],
                                     start=(c == 0), stop=(c == DC - 1))
                sqt = []
                for c in range(DC):
                    sq = tmpp.tile([P, T], f32r, tag="lnsq")
                    nc.vector.tensor_mul(sq[:], F(xsrc[:, c, :]),
                                         F(xsrc[:, c, :]))
                    sqt.append(sq)
                for c in range(DC):
                    nc.tensor.matmul(psum_sq[:], ones_col[:], sqt[c][:],
                                     start=(c == 0), stop=(c == DC - 1))
                mu_f = small.tile([1, T], f32, tag="ln_mu")
                nc.vector.tensor_scalar_mul(mu_f[:], psum_mu[:], 1.0 / D)
                mu2_f = small.tile([1, T], f32, tag="ln_mu2")
                nc.vector.tensor_tensor(mu2_f[:], mu_f[:], mu_f[:], ALU.mult)
                var_f = small.tile([1, T], f32, tag="ln_var")
                nc.vector.scalar_tensor_tensor(
                    var_f[:], psum_sq[:], 1.0 / D, mu2_f[:],
                    op0=ALU.mult, op1=ALU.subtract)
                rs_f = small.tile([1, T], f32, tag="ln_rs")
                nc.scalar.activation(rs_f[:], var_f[:], AF.Abs_reciprocal_sqrt,
                                     bias=eps_t[:])
                rs_r = small.tile([1, T], f32r, tag="ln_rs_r")
                nc.vector.tensor_copy(rs_r[:], rs_f[:])
                mrs_r = small.tile([1, T], f32r, tag="ln_mrs_r")
                nc.vector.tensor_tensor(mrs_r[:], mu_f[:], rs_f[:], ALU.mult)
                for c in range(DC):
                    bcA = ps_bc.tile([P, T], f32, tag="ln_bcA")
                    bcB = ps_bc.tile([P, T], f32, tag="ln_bcB")
                    gsl = g_row[:, c * P:(c + 1) * P]
                    bsl = bn_row[:, c * P:(c + 1) * P]
                    nc.tensor.matmul(bcA[:], gsl, rs_r[:], start=True, stop=True)
                    nc.tensor.matmul(bcB[:], gsl, mrs_r[:], start=True, stop=False)
                    nc.tensor.matmul(bcB[:], bsl, ones_row[:], start=False, stop=True)
                    tmp = tmpp.tile([P, T], f32, tag="ln_tmp")
                    nc.vector.tensor_tensor(tmp[:], F(xsrc[:, c, :]), bcA[:],
                                            ALU.mult)
                    nc.vector.tensor_tensor(dst[:, c, :], tmp[:], bcB[:],
                                            ALU.subtract)

            for b in range(BPC):
                # ================= phase A: load + Q/V projections ======
                qin = act.tile([P, DC, T], f32r, tag="act")
                nc.sync.dma_start(qin[:], qT_d.ap()[b].rearrange(
                    "(c p) t -> p c t", p=P))
                kin = bigk.tile([P, DC, S], bf16, tag="kin")
                nc.gpsimd.dma_start(kin[:], kT_d.ap()[b].rearrange(
                    "(c p) s -> p c s", p=P))
                wv_sb = wvp.tile([P, DC, D], bf16, tag="wv")
                nc.gpsimd.dma_start(wv_sb[:], wv_d.ap().rearrange(
                    "(k p) o -> p k o", p=P))

                qt = qtp.tile([P, DC, T], bf16, tag="qt")
                with tc.tile_pool(name="psA", bufs=2, space="PSUM") as psA:
                    for mo in range(DC):
                        wq_sl = wstream.tile([P, DC, P], f32r, tag="wq_sl")
                        nc.sync.dma_start(wq_sl[:], wq_d.ap().rearrange(
                            "(k p) o -> p k o", p=P)[:, :, mo * P:(mo + 1) * P])
                        ps = psA.tile([P, T], f32, tag="psA")
                        for ki in range(DC):
                            nc.tensor.matmul(ps[:], wq_sl[:, ki, :],
                                             qin[:, ki, :],
                                             start=(ki == 0), stop=(ki == DC - 1))
                        nc.vector.tensor_scalar_add(qt[:, mo, :], ps[:],
                                                    bq_pc[:, mo:mo + 1])

                    v_sb = vpool.tile([P, SC, H, HD + 1], bf16, tag="v")
                    nc.vector.tensor_copy(
                        v_sb[:, :, :, HD:HD + 1],
                        onesc_f[:].to_broadcast([P, SC, H, 1]))
                    bv_bc = None
                    if use_bv:
                        bv_bc = small.tile([P, D], f32, tag="bv_bc")
                        for half in range(2):
                            ps_bv = psA.tile([P, 384], f32, tag="psA")
                            nc.tensor.matmul(
                                ps_bv[:], ones_row[:, 0:P],
                                bv_row[:, half * 384:(half + 1) * 384],
                                start=True, stop=True)
                            nc.vector.tensor_copy(
                                bv_bc[:, half * 384:(half + 1) * 384], ps_bv[:])
                    for so in range(SC):
                        for half in range(2):
                            ps = psA.tile([P, 384], f32, tag="psA")
                            for ki in range(DC):
                                nc.tensor.matmul(
                                    ps[:],
                                    kin[:, ki, so * P:(so + 1) * P],
                                    wv_sb[:, ki, half * 384:(half + 1) * 384],
                                    start=(ki == 0), stop=(ki == DC - 1))
                            dstv = v_sb[:, so, half * 6:(half + 1) * 6, 0:HD]
                            if use_bv:
                                nc.vector.tensor_tensor(
                                    dstv, ps[:],
                                    bv_bc[:, half * 384:(half + 1) * 384],
                                    ALU.add)
                            else:
                                nc.vector.tensor_copy(dstv, ps[:])

                # ================= phase B: attention ====================
                ctxT = act.tile([P, DC, T], bf16, tag="act")

                def attn_kproj(hp, kin, psK):
                    wk_sl = wstream.tile([P, DC, P], bf16, tag="wk_sl")
                    nc.gpsimd.dma_start(wk_sl[:], wk_d.ap().rearrange(
                        "(k p) o -> p k o", p=P)[:, :, hp * P:(hp + 1) * P])
                    ktch = ktc.tile([P, S], bf16, tag="ktc")
                    for no in range(4):
                        ps = psK.tile([P, T], f32, tag="psK")
                        for ki in range(DC):
                            nc.tensor.matmul(
                                ps[:], wk_sl[:, ki, :],
                                kin[:, ki, no * T:(no + 1) * T],
                                start=(ki == 0), stop=(ki == DC - 1))
                        nc.vector.tensor_scalar_add(
                            ktch[:, no * T:(no + 1) * T], ps[:],
                            bk_pc[:, hp:hp + 1])
                    return ktch

                def attn_scores_ctx(hp, so2, ktch, qt, v_sb, ps_ctx, psSC):
                    scs = []
                    for hh in range(2):
                        base = hh * HD
                        ps_sc = psSC.tile([P, 2 * T], f32, tag="psSC",
                                          name=f"ps_sc{hh}")
                        for j in range(2):
                            so = so2 + j
                            nc.tensor.matmul(
                                ps_sc[:, j * T:(j + 1) * T],
                                ktch[base:base + HD, so * P:(so + 1) * P],
                                qt[base:base + HD, hp, :],
                                start=True, stop=True)
                        scs.append(ps_sc)
                    exs = []
                    for hh in range(2):
                        ex = expp.tile([P, 2 * T], bf16, tag="exp",
                                       name=f"ex{hh}")
                        nc.scalar.activation(ex[:], scs[hh][:], AF.Exp)
                        exs.append(ex)
                    for hh in range(2):
                        h = 2 * hp + hh
                        for j in range(2):
                            so = so2 + j
                            nc.tensor.matmul(
                                ps_ctx[hh][:], v_sb[:, so, h, :],
                                exs[hh][:, j * T:(j + 1) * T],
                                start=(so == 0), stop=(so == SC - 1))

                def attn_evict(hp, hh, ps_ctx, ctxT, psBC):
                    base = hh * HD
                    rden_f = tmpp.tile([1, T], f32, tag="rden_f")
                    act_reciprocal(rden_f[:], ps_ctx[hh][HD:HD + 1, :])
                    rden_bf = tmpp.tile([1, T], bf16, tag="rden_bf")
                    nc.vector.tensor_copy(rden_bf[:], rden_f[:])
                    ps_bc = psBC.tile([HD, T], f32, tag="psBC")
                    nc.tensor.matmul(ps_bc[:], ones64_bf[:],
                                     rden_bf[:], start=True, stop=True)
                    bc_sb = tmpp.tile([HD, T], f32, tag="bc_sb")
                    nc.vector.tensor_copy(bc_sb[:], ps_bc[:])
                    nc.vector.tensor_tensor(
                        ctxT[base:base + HD, hp, :],
                        ps_ctx[hh][0:HD, :], bc_sb[:], ALU.mult)

                with (
                    tc.tile_pool(name="psK", bufs=1, space="PSUM") as psK,
                    tc.tile_pool(name="psSC", bufs=2, space="PSUM") as psSC,
                    tc.tile_pool(name="psCTX", bufs=2, space="PSUM") as psCTX,
                    tc.tile_pool(name="psBC", bufs=1, space="PSUM") as psBC,
                ):
                    for hp in range(DC):
                        ktch = attn_kproj(hp, kin, psK)
                        ps_ctx = [psCTX.tile([HD + 1, T], f32, tag="psCTX",
                                            name=f"ps_ctx{i}")
                                  for i in range(2)]
                        for so2 in range(0, SC, 2):
                            attn_scores_ctx(hp, so2, ktch, qt, v_sb,
                                            ps_ctx, psSC)
                        for hh in range(2):
                            attn_evict(hp, hh, ps_ctx, ctxT, psBC)

                # ================= phase C: out_proj + residual ==========
                xT = act.tile([P, DC, T], f32r, tag="act")
                with tc.tile_pool(name="psC", bufs=2, space="PSUM") as psC:
                    for mo in range(DC):
                        wo_sl = wstream.tile([P, DC, P], bf16, tag="wo_sl")
                        nc.gpsimd.dma_start(wo_sl[:], wo_d.ap().rearrange(
                            "(k p) o -> p k o", p=P)[:, :, mo * P:(mo + 1) * P])
                        ps = psC.tile([P, T], f32, tag="psC")
                        for ki in range(DC):
                            nc.tensor.matmul(ps[:], wo_sl[:, ki, :],
                                             ctxT[:, ki, :],
                                             start=(ki == 0), stop=(ki == DC - 1))
                        nc.vector.scalar_tensor_tensor(
                            xT[:, mo, :], ps[:], bo_pc[:, mo:mo + 1],
                            F(qin[:, mo, :]), op0=ALU.add, op1=ALU.add)

                # ================= phase D: LN2 ==========================
                hT = act.tile([P, DC, T], f32r, tag="act")
                with (
                    tc.tile_pool(name="psST", bufs=1, space="PSUM") as psST,
                    tc.tile_pool(name="psLB", bufs=2, space="PSUM") as psLB,
                ):
                    ln_pass(xT, hT, ln2g, ln2bn, (psST, psLB))

                # ================= phase E: MLP (fused fc1->gelu->fc2) ===
                x2T = act.tile([P, DC, T], f32r, tag="act")
                with (
                    tc.tile_pool(name="psF1", bufs=2, space="PSUM") as psF1,
                    tc.tile_pool(name="psF2", bufs=6, space="PSUM") as psF2,
                ):
                    ps_f2 = [psF2.tile([P, T], f32, tag="psF2", name=f"ps_f2_{i}")
                             for i in range(DC)]
                    for fo in range(FFC):
                        f1_sl = fstream.tile([P, DC, P], f32r, tag="f1_sl")
                        nc.sync.dma_start(f1_sl[:], fc1_d.ap().rearrange(
                            "(k p) f -> p k f", p=P)[:, :, fo * P:(fo + 1) * P])
                        f2_sl = fstream.tile([P, D], f32r, tag="f2_sl")
                        nc.sync.dma_start(f2_sl[:], fc2_d.ap().rearrange(
                            "(ko p) o -> p ko o", p=P)[:, fo, :])
                        ps1 = psF1.tile([P, T], f32, tag="psF1")
                        for ki in range(DC):
                            nc.tensor.matmul(ps1[:], f1_sl[:, ki, :],
                                             hT[:, ki, :],
                                             start=(ki == 0), stop=(ki == DC - 1))
                        mch = mchunkp.tile([P, T], f32r, tag="mch")
                        nc.scalar.activation(mch[:], ps1[:], AF.Gelu,
                                             bias=f1b_pc[:, fo:fo + 1])
                        for mo in range(DC):
                            nc.tensor.matmul(
                                ps_f2[mo][:], f2_sl[:, mo * P:(mo + 1) * P],
                                mch[:],
                                start=(fo == 0), stop=(fo == FFC - 1))
                    for mo in range(DC):
                        nc.vector.scalar_tensor_tensor(
                            x2T[:, mo, :], ps_f2[mo][:], f2b_pc[:, mo:mo + 1],
                            F(xT[:, mo, :]), op0=ALU.add, op1=ALU.add)

                # ================= phase F: LNp + store ==================
                outT = act.tile([P, DC, T], f32, tag="act")
                with (
                    tc.tile_pool(name="psST2", bufs=1, space="PSUM") as psST2,
                    tc.tile_pool(name="psLB2", bufs=2, space="PSUM") as psLB2,
                ):
                    ln_pass(x2T, outT, lnpg, lnpbn, (psST2, psLB2))
                nc.sync.dma_start(
                    out_d.ap()[b].rearrange("(c p) t -> p c t", p=P), outT[:])

    nc.compile()
    return nc


def _get_nc(use_bv: bool):
    key = ("nc", use_bv)
    if key not in _cached:
        _cached[key] = _build(use_bv)
    return _cached[key]


def kernel(query, key, wq, bq, wk, bk, wv, bv, wo, bo,
           ln2_g, ln2_b, fc1_w, fc1_b, fc2_w, fc2_b, lnp_g, lnp_b):
    from concourse.bass_utils import run_bass_kernel_spmd

    f = np.float32
    c = np.ascontiguousarray
    query = np.asarray(query, f)
    key = np.asarray(key, f)
    use_bv = bool(np.any(np.asarray(bv)))
    nc = _get_nc(use_bv)

    shared = {
        "wq_t": c(np.asarray(wq, f).T * np.float32(SCALE)),
        "wk_t": c(np.asarray(wk, f).T),
        "wv_t": c(np.asarray(wv, f).T),
        "wo_t": c(np.asarray(wo, f).T),
        "fc1_t": c(np.asarray(fc1_w, f).T),
        "fc2_t": c(np.asarray(fc2_w, f).T),
        "bqv": c(np.asarray(bq, f) * np.float32(SCALE)),
        "bkv": c(np.asarray(bk, f)),
        "bvv": c(np.asarray(bv, f).reshape(1, D)),
        "bov": c(np.asarray(bo, f)),
        "f1b": c(np.asarray(fc1_b, f)),
        "f2b": c(np.asarray(fc2_b, f)),
        "ln2g": c(np.asarray(ln2_g, f).reshape(1, D)),
        "ln2bn": c(-np.asarray(ln2_b, f).reshape(1, D)),
        "lnpg": c(np.asarray(lnp_g, f).reshape(1, D)),
        "lnpbn": c(-np.asarray(lnp_b, f).reshape(1, D)),
        "ones_col": np.ones((P, 1), f),
        "ones_row": np.ones((1, T), f),
    }
    in_maps = []
    for core in range(NCORES):
        sl = slice(core * BPC, (core + 1) * BPC)
        m = dict(shared)
        m["qT"] = c(query[sl].transpose(0, 2, 1))
        m["kT"] = c(key[sl].transpose(0, 2, 1))
        in_maps.append(m)

    res = run_bass_kernel_spmd(nc, in_maps, core_ids=list(range(NCORES)))
    kernel._last_result = res
    out = np.concatenate([r["out"] for r in res.results], axis=0)
    return c(out.transpose(0, 2, 1))



# revision 35
# speedup vs baseline: 1.2245x; 1.2245x over previous
"""GroupViT cross-attention layer on 8 TRN2 NeuronCores.

Data-parallel over batch (16 -> 2 per core, zero collectives),
feature-major layout: activations [feature(partition), token(free)],
weights host-transposed+pre-tiled so matmuls contract over partitions.

v2: fp8(e4m3) + DoubleRow for all contract-768/3072 matmuls (Q/K/V/O
projections, fc1, fc2: contraction pairs = adjacent 128-feature chunks,
which already sit along the free dim -> no layout change) and for the
ctx matmul (pairs = adjacent key-token blocks, matching the [P,2T] exp
output layout). Scores stay bf16 (contract=64: DoubleRow can't help).
Residual / LayerNorm path stays f32. Host quantizes weights + q/k inputs
to e4m3 with power-of-2 scales; descale factors fold into the existing
eviction ops (ACT scale arg of exp/gelu, DVE tensor_scalar dual-op).

Softmax: scores O(+-2) so exp needs no max subtraction; denominators
come free from a ones column appended to V; reciprocal on DVE (keeps
the ACT table loaded with Exp); per-head 1/den broadcast via one k=2
matmul with a constant selector stationary.

LayerNorm over the partition (feature) dim: mean/sq sums via ones-column
matmuls; per chunk the two rank-1 broadcasts (g*rs and g*mu*rs - b) are
produced by ONE k=3 matmul into a [P, 2T] psum: lhsT rows [g, g, -b],
moving rows [rs|0, 0|mu*rs, 0|ones].
"""

import os

import numpy as np

B, T, S, D, H, HD, FF = 16, 512, 2048, 768, 12, 64, 3072
NCORES = 8
BPC = B // NCORES      # batches per core
P = 128
DC = D // P            # 6 feature chunks
SC = S // P            # 16 key-token chunks
FFC = FF // P          # 24
EPS = 1e-5
SCALE = HD ** -0.5

# power-of-2 fp8 scales (host multiplies in, kernel divides out)
SQ = 256.0             # wq (incl. SCALE)
SK = 32.0              # wk (ktch stored fp8 at this scale: max |k|*32 < 240)
SV = 64.0              # wv
SO = 64.0              # wo
SCTX = 32.0            # ctx eviction
SF1 = 64.0             # fc1
SF2 = 64.0             # fc2

MLP_FP8 = os.environ.get("KMLP", "fp8") == "fp8"

_cached = {}


def _build(use_bv: bool, use_bo: bool, use_f2b: bool, mlp_fp8: bool):
    import concourse.bacc as bacc
    import concourse.tile as tile
    import concourse.mybir as mybir

    f32 = mybir.dt.float32
    f32r = mybir.dt.float32r
    bf16 = mybir.dt.bfloat16
    fp8 = mybir.dt.float8e4
    AF = mybir.ActivationFunctionType
    ALU = mybir.AluOpType
    DR = mybir.MatmulPerfMode.DoubleRow

    nc = bacc.Bacc("TRN2", target_bir_lowering=False, debug=False,
                   num_devices=NCORES)

    # ---- DRAM I/O (per-core shapes) ----
    qT_d = nc.dram_tensor("qT", [BPC, D, T], f32r, kind="ExternalInput")
    qT8_d = nc.dram_tensor("qT8", [BPC, D, T], fp8, kind="ExternalInput")
    kT8_d = nc.dram_tensor("kT8", [BPC, D, S], fp8, kind="ExternalInput")
    wq8_d = nc.dram_tensor("wq8", [P, DC, DC, P], fp8, kind="ExternalInput")
    wk8_d = nc.dram_tensor("wk8", [P, DC, DC, P], fp8, kind="ExternalInput")
    wv8_d = nc.dram_tensor("wv8", [P, DC, D], fp8, kind="ExternalInput")
    wo8_d = nc.dram_tensor("wo8", [P, DC, DC, P], fp8, kind="ExternalInput")
    mlp_dt = fp8 if mlp_fp8 else bf16
    fc18_d = nc.dram_tensor("fc18", [P, FFC, DC, P], mlp_dt,
                            kind="ExternalInput")
    fc28_d = nc.dram_tensor("fc28", [P, FFC, D], mlp_dt,
                            kind="ExternalInput")
    bq_d = nc.dram_tensor("bqv", [P, DC], f32, kind="ExternalInput")
    bk_d = nc.dram_tensor("bkv", [P, DC], f32, kind="ExternalInput")
    bv_d = nc.dram_tensor("bvv", [1, D], f32r, kind="ExternalInput")
    bo_d = nc.dram_tensor("bov", [P, DC], f32, kind="ExternalInput")
    f1b_d = nc.dram_tensor("f1b", [P, FFC], f32, kind="ExternalInput")
    f2b_d = nc.dram_tensor("f2b", [P, DC], f32, kind="ExternalInput")
    ln2g3_d = nc.dram_tensor("ln2g3", [3, DC, P], f32r, kind="ExternalInput")
    lnpg3_d = nc.dram_tensor("lnpg3", [3, DC, P], f32r, kind="ExternalInput")
    ones_col_d = nc.dram_tensor("ones_col", [P, 1], f32r, kind="ExternalInput")
    ones_row_d = nc.dram_tensor("ones_row", [1, T], f32r, kind="ExternalInput")
    sel8_d = nc.dram_tensor("sel8v", [1, P], f32r, kind="ExternalInput")
    out_d = nc.dram_tensor("out", [BPC, D, T], f32, kind="ExternalOutput")

    def F(ap):
        return ap.bitcast(f32)

    with tile.TileContext(nc) as tc:
        with (
            tc.tile_pool(name="act", bufs=4) as act,        # f32r [P,DC,T]
            tc.tile_pool(name="act8", bufs=4 if mlp_fp8 else 3) as act8,
            tc.tile_pool(name="hbf", bufs=2) as hbf,        # bf16 hT (cfg A)
            tc.tile_pool(name="kin", bufs=2) as kinp,
            tc.tile_pool(name="vpool", bufs=2) as vpool,
            tc.tile_pool(name="qtp", bufs=2) as qtp,
            tc.tile_pool(name="ktc", bufs=2) as ktc,
            tc.tile_pool(name="wres", bufs=1) as wres,
            tc.tile_pool(name="fstream", bufs=2) as fstream,
            tc.tile_pool(name="expp", bufs=3) as expp,
            tc.tile_pool(name="mchunk", bufs=2) as mchunkp,
            tc.tile_pool(name="tmp", bufs=2) as tmpp,
            tc.tile_pool(name="ln1", bufs=3) as ln1p,
            tc.tile_pool(name="sqp", bufs=2) as sqp,
            tc.tile_pool(name="rsp", bufs=2) as rsp,
            tc.tile_pool(name="small", bufs=1) as small,
        ):
            # ---- persistent small tiles ----
            ones_col = small.tile([P, 1], f32r, tag="ones_col")
            nc.sync.dma_start(ones_col[:], ones_col_d.ap())
            ones_row = small.tile([1, T], f32r, tag="ones_row")
            nc.sync.dma_start(ones_row[:], ones_row_d.ap())
            eps_t = small.tile([1, 1], f32, tag="eps")
            nc.vector.memset(eps_t[:], EPS)
            onesc_f = small.tile([P, 1], f32, tag="onesc_f")
            nc.vector.memset(onesc_f[:], 1.0)

            # broadcast row for per-head 1/den: bc = sel8 (x) rden  (all rows
            # equal SCTX*rden; the eviction mult only reads 64 of them)
            sel8 = small.tile([1, P], f32r, tag="sel8")
            nc.sync.dma_start(sel8[:], sel8_d.ap())

            bq_pc = small.tile([P, DC], f32, tag="bq_pc")
            nc.sync.dma_start(bq_pc[:], bq_d.ap())
            bk_pc = small.tile([P, DC], f32, tag="bk_pc")
            nc.sync.dma_start(bk_pc[:], bk_d.ap())
            bo_pc = small.tile([P, DC], f32, tag="bo_pc")
            nc.sync.dma_start(bo_pc[:], bo_d.ap())
            f1b_pc = small.tile([P, FFC], f32, tag="f1b_pc")
            nc.sync.dma_start(f1b_pc[:], f1b_d.ap())
            f2b_pc = small.tile([P, DC], f32, tag="f2b_pc")
            nc.sync.dma_start(f2b_pc[:], f2b_d.ap())
            ln2g1 = small.tile([1, DC, P], f32r, tag="ln2g1")
            nc.sync.dma_start(ln2g1[:], ln2g3_d.ap()[0:1])
            ln2g2 = small.tile([2, DC, P], f32r, tag="ln2g2")
            nc.sync.dma_start(ln2g2[:], ln2g3_d.ap()[1:3])
            lnpg1 = small.tile([1, DC, P], f32r, tag="lnpg1")
            nc.sync.dma_start(lnpg1[:], lnpg3_d.ap()[0:1])
            lnpg2 = small.tile([2, DC, P], f32r, tag="lnpg2")
            nc.sync.dma_start(lnpg2[:], lnpg3_d.ap()[1:3])

            # ---- resident attention weights (fp8, 2.4MB) ----
            wq8 = wres.tile([P, DC, DC, P], fp8, tag="wq8")
            nc.sync.dma_start(wq8[:], wq8_d.ap())
            wk8 = wres.tile([P, DC, DC, P], fp8, tag="wk8")
            nc.sync.dma_start(wk8[:], wk8_d.ap())
            wv8 = wres.tile([P, DC, D], fp8, tag="wv8")
            nc.sync.dma_start(wv8[:], wv8_d.ap())
            wo8 = wres.tile([P, DC, DC, P], fp8, tag="wo8")
            nc.sync.dma_start(wo8[:], wo8_d.ap())

            bv_bc = None
            if use_bv:
                bv_row = small.tile([1, D], f32r, tag="bv_row")
                nc.sync.dma_start(bv_row[:], bv_d.ap())
                bv_bc = small.tile([P, D], f32, tag="bv_bc")
                with tc.tile_pool(name="psBV", bufs=2, space="PSUM") as psBV:
                    for half in range(2):
                        ps_bv = psBV.tile([P, 384], f32, tag="psBV")
                        nc.tensor.matmul(
                            ps_bv[:], ones_row[:, 0:P],
                            bv_row[:, half * 384:(half + 1) * 384],
                            start=True, stop=True)
                        nc.vector.tensor_copy(
                            bv_bc[:, half * 384:(half + 1) * 384], ps_bv[:])

            # per-batch state
            qin8 = [None] * BPC
            kin8 = [None] * BPC
            qt = [None] * BPC
            v_sb = [None] * BPC
            ctxT = [None] * BPC
            qin = [None] * BPC
            xT = [None] * BPC
            hT = [None] * BPC
            x2T = [None] * BPC

            # ============ phase A: loads + Q/V projections ============
            def phaseA(b, psA):
                qin8[b] = act8.tile([P, DC, T], fp8, tag="act8",
                                    name=f"qin8_{b}")
                nc.sync.dma_start(qin8[b][:], qT8_d.ap()[b].rearrange(
                    "(c p) t -> p c t", p=P))
                kin8[b] = kinp.tile([P, DC, S], fp8, tag="kin",
                                    name=f"kin8_{b}")
                nc.sync.dma_start(kin8[b][:], kT8_d.ap()[b].rearrange(
                    "(c p) s -> p c s", p=P))

                qt[b] = qtp.tile([P, DC, T], fp8, tag="qt", name=f"qt_{b}")
                for mo in range(DC):
                    ps = psA.tile([P, T], f32, tag="psA")
                    for cp in range(DC // 2):
                        nc.tensor.matmul(
                            ps[:], wq8[:, mo, 2 * cp:2 * cp + 2, :],
                            qin8[b][:, 2 * cp:2 * cp + 2, :],
                            start=(cp == 0), stop=(cp == DC // 2 - 1),
                            perf_mode=DR)
                    nc.vector.tensor_scalar_add(qt[b][:, mo, :], ps[:],
                                                bq_pc[:, mo:mo + 1])

                # last dim padded 65->80: DoubleRow LDWEIGHTS needs the pair
                # stride (H*80=960 bytes) to be 16-aligned. col 64 = ones
                # (denominator row), cols 65..79 junk (psum rows never read).
                v_sb[b] = vpool.tile([P, SC, H, 80], fp8, tag="v",
                                     name=f"v_{b}")
                nc.vector.tensor_copy(
                    v_sb[b][:, :, :, HD:HD + 1],
                    onesc_f[:].to_broadcast([P, SC, H, 1]))
                for so in range(SC):
                    for half in range(2):
                        ps = psA.tile([P, 384], f32, tag="psA")
                        for cp in range(DC // 2):
                            nc.tensor.matmul(
                                ps[:],
                                kin8[b][:, 2 * cp:2 * cp + 2,
                                        so * P:(so + 1) * P],
                                wv8[:, 2 * cp:2 * cp + 2,
                                    half * 384:(half + 1) * 384],
                                start=(cp == 0), stop=(cp == DC // 2 - 1),
                                perf_mode=DR)
                        dstv = v_sb[b][:, so, half * 6:(half + 1) * 6, 0:HD]
                        if use_bv:
                            nc.vector.scalar_tensor_tensor(
                                dstv, ps[:], 1.0 / SV,
                                bv_bc[:, half * 384:(half + 1) * 384],
                                op0=ALU.mult, op1=ALU.add)
                        else:
                            nc.vector.tensor_scalar_mul(dstv, ps[:], 1.0 / SV)

            # ============ phase B: attention ============
            def phaseB(b, pools):
                psAux, psSC, psCTX = pools
                ctxT[b] = act8.tile([P, DC, T], fp8, tag="act8",
                                    name=f"ctxT_{b}")
                for hp in range(DC):
                    # K projection for heads (2hp, 2hp+1)
                    ktch = ktc.tile([P, S], fp8, tag="ktc")
                    for no in range(4):
                        ps = psAux.tile([P, T], f32, tag="psAux")
                        for cp in range(DC // 2):
                            nc.tensor.matmul(
                                ps[:], wk8[:, hp, 2 * cp:2 * cp + 2, :],
                                kin8[b][:, 2 * cp:2 * cp + 2,
                                        no * T:(no + 1) * T],
                                start=(cp == 0), stop=(cp == DC // 2 - 1),
                                perf_mode=DR)
                        nc.vector.tensor_scalar_add(
                            ktch[:, no * T:(no + 1) * T], ps[:],
                            bk_pc[:, hp:hp + 1])

                    ps_ctx = [psCTX.tile([80, T], f32, tag="psCTX",
                                         name=f"ps_ctx{i}")
                              for i in range(2)]
                    for so2 in range(0, SC, 2):
                        exs = []
                        for hh in range(2):
                            base = hh * HD
                            ps_sc = psSC.tile([P, 2 * T], f32, tag="psSC",
                                              name=f"ps_sc{hh}")
                            for j in range(2):
                                so = so2 + j
                                nc.tensor.matmul(
                                    ps_sc[:, j * T:(j + 1) * T],
                                    ktch[base:base + HD,
                                         so * P:(so + 1) * P],
                                    qt[b][base:base + HD, hp, :],
                                    start=True, stop=True)
                            ex = expp.tile([P, 2, T], fp8, tag="exp",
                                           name=f"ex{hh}")
                            nc.scalar.activation(
                                ex[:], ps_sc[:], AF.Exp,
                                scale=1.0 / (SQ * SK))
                            exs.append(ex)
                        for hh in range(2):
                            h = 2 * hp + hh
                            nc.tensor.matmul(
                                ps_ctx[hh][:],
                                v_sb[b][:, so2:so2 + 2, h, :],
                                exs[hh][:],
                                start=(so2 == 0), stop=(so2 == SC - 2),
                                perf_mode=DR)

                    # eviction: normalize by denominator (row HD), scale SCTX
                    rden = [tmpp.tile([1, T], f32r, tag="rden",
                                      name=f"rden{i}") for i in range(2)]
                    with nc.allow_low_precision(reason="f32r rden"):
                        for hh in range(2):
                            nc.vector.reciprocal(rden[hh][:],
                                                 ps_ctx[hh][HD:HD + 1, :])
                    for hh in range(2):
                        base = hh * HD
                        ps_bc = psAux.tile([P, T], f32, tag="psAux")
                        nc.tensor.matmul(ps_bc[:], sel8[:], rden[hh][:],
                                         start=True, stop=True)
                        bc_sb = tmpp.tile([P, T], f32, tag="bc_sb")
                        nc.vector.tensor_copy(bc_sb[:], ps_bc[:])
                        nc.vector.tensor_tensor(
                            ctxT[b][base:base + HD, hp, :],
                            ps_ctx[hh][0:HD, :], bc_sb[0:HD, :], ALU.mult)

            # ============ phase C: out_proj + residual ============
            def phaseC(b, psC):
                qin[b] = act.tile([P, DC, T], f32r, tag="act",
                                  name=f"qin_{b}")
                nc.sync.dma_start(qin[b][:], qT_d.ap()[b].rearrange(
                    "(c p) t -> p c t", p=P))
                if use_bo:
                    qin_bo = act.tile([P, DC, T], f32r, tag="act",
                                      name=f"qinbo_{b}")
                    for mo in range(DC):
                        nc.vector.tensor_scalar_add(
                            qin_bo[:, mo, :], F(qin[b][:, mo, :]),
                            bo_pc[:, mo:mo + 1])
                else:
                    qin_bo = qin[b]
                xT[b] = act.tile([P, DC, T], f32r, tag="act", name=f"xT_{b}")
                for mo in range(DC):
                    ps = psC.tile([P, T], f32, tag="psC")
                    for cp in range(DC // 2):
                        nc.tensor.matmul(
                            ps[:], wo8[:, mo, 2 * cp:2 * cp + 2, :],
                            ctxT[b][:, 2 * cp:2 * cp + 2, :],
                            start=(cp == 0), stop=(cp == DC // 2 - 1),
                            perf_mode=DR)
                    nc.vector.scalar_tensor_tensor(
                        xT[b][:, mo, :], ps[:], 1.0 / (SCTX * SO),
                        F(qin_bo[:, mo, :]), op0=ALU.mult, op1=ALU.add)

            # ============ LayerNorm over feature(partition) dim ============
            def ln_pass(xsrc, dst, g1, g2, ps_scope):
                """dst[:,c,:] = (x - mu)*rsqrt(var+eps)*g + b via one k=3
                matmul per chunk producing [g*rs | g*mu*rs - b]."""
                ps_st, ps_lb = ps_scope
                psum_mu = ps_st.tile([1, T], f32, tag="st_mu")
                psum_sq = ps_st.tile([1, T], f32, tag="st_sq")
                for c in range(DC):
                    nc.tensor.matmul(psum_mu[:], ones_col[:], xsrc[:, c, :],
                                     start=(c == 0), stop=(c == DC - 1))
                sqt = []
                for c in range(DC):
                    sq = sqp.tile([P, T], f32r, tag="lnsq")
                    nc.vector.tensor_mul(sq[:], F(xsrc[:, c, :]),
                                         F(xsrc[:, c, :]))
                    sqt.append(sq)
                for c in range(DC):
                    nc.tensor.matmul(psum_sq[:], ones_col[:], sqt[c][:],
                                     start=(c == 0), stop=(c == DC - 1))
                mu_f = ln1p.tile([1, T], f32, tag="ln1t", name="ln_mu")
                nc.vector.tensor_scalar_mul(mu_f[:], psum_mu[:], 1.0 / D)
                mu2_f = ln1p.tile([1, T], f32, tag="ln1t", name="ln_mu2")
                nc.vector.tensor_tensor(mu2_f[:], mu_f[:], mu_f[:], ALU.mult)
                var_f = ln1p.tile([1, T], f32, tag="ln1t", name="ln_var")
                nc.vector.scalar_tensor_tensor(
                    var_f[:], psum_sq[:], 1.0 / D, mu2_f[:],
                    op0=ALU.mult, op1=ALU.subtract)
                # per chunk c two broadcast matmuls into one [P,2T] psum:
                #   cols 0:T   <- g (x) rs           (k=1)
                #   cols T:2T  <- g (x) mu*rs - b    (k=2, rows [mrs, ones])
                rs_t = rsp.tile([1, T], f32r, tag="rs_t")
                nc.scalar.activation(rs_t[:], var_f[:],
                                     AF.Abs_reciprocal_sqrt, bias=eps_t[:])
                rsm2 = rsp.tile([2, T], f32r, tag="rsm2")
                nc.vector.tensor_tensor(rsm2[0:1, :], mu_f[:],
                                        F(rs_t[:]), ALU.mult)
                # engines can't write partition 1; DMA can
                nc.sync.dma_start(rsm2[1:2, :], ones_row_d.ap())
                for c in range(DC):
                    ps_b = ps_lb.tile([P, 2 * T], f32, tag="ln_bc")
                    nc.tensor.matmul(ps_b[:, 0:T], g1[:, c, :], rs_t[:],
                                     start=True, stop=True)
                    nc.tensor.matmul(ps_b[:, T:2 * T], g2[:, c, :],
                                     rsm2[:], start=True, stop=True)
                    tmp = tmpp.tile([P, T], f32, tag="ln_tmp")
                    nc.vector.tensor_tensor(tmp[:], F(xsrc[:, c, :]),
                                            ps_b[:, 0:T], ALU.mult)
                    nc.vector.tensor_tensor(dst[:, c, :], tmp[:],
                                            ps_b[:, T:2 * T], ALU.subtract)

            def phaseD(b, ps_scope):
                if mlp_fp8:
                    hT[b] = act8.tile([P, DC, T], fp8, tag="act8",
                                      name=f"hT_{b}")
                else:
                    hT[b] = hbf.tile([P, DC, T], bf16, tag="hbf",
                                     name=f"hT_{b}")
                ln_pass(xT[b], hT[b], ln2g1, ln2g2, ps_scope)

            # ============ phase E: MLP ============
            def phaseE(b, psF1, psF2):
                if use_f2b:
                    xT_f2b = act.tile([P, DC, T], f32r, tag="act",
                                      name=f"xtf2b_{b}")
                    for mo in range(DC):
                        nc.vector.tensor_scalar_add(
                            xT_f2b[:, mo, :], F(xT[b][:, mo, :]),
                            f2b_pc[:, mo:mo + 1])
                else:
                    xT_f2b = xT[b]
                x2T[b] = act.tile([P, DC, T], f32r, tag="act",
                                  name=f"x2T_{b}")
                ps_f2 = [psF2.tile([P, T], f32, tag="psF2",
                                   name=f"ps_f2_{i}") for i in range(DC)]
                for fo2 in range(FFC // 2):
                    mch = mchunkp.tile([P, 2, T], mlp_dt, tag="mch")
                    f2_sl = fstream.tile([P, 2, D], mlp_dt, tag="f2_sl")
                    nc.sync.dma_start(
                        f2_sl[:], fc28_d.ap()[:, 2 * fo2:2 * fo2 + 2, :])
                    for par in range(2):
                        fo = 2 * fo2 + par
                        f1_sl = fstream.tile([P, DC, P], mlp_dt, tag="f1_sl")
                        nc.sync.dma_start(f1_sl[:], fc18_d.ap()[:, fo, :, :])
                        ps1 = psF1.tile([P, T], f32, tag="psF1")
                        if mlp_fp8:
                            for cp in range(DC // 2):
                                nc.tensor.matmul(
                                    ps1[:], f1_sl[:, 2 * cp:2 * cp + 2, :],
                                    hT[b][:, 2 * cp:2 * cp + 2, :],
                                    start=(cp == 0),
                                    stop=(cp == DC // 2 - 1), perf_mode=DR)
                        else:
                            for ki in range(DC):
                                nc.tensor.matmul(
                                    ps1[:], f1_sl[:, ki, :], hT[b][:, ki, :],
                                    start=(ki == 0), stop=(ki == DC - 1))
                        nc.scalar.activation(mch[:, par, :], ps1[:], AF.Gelu,
                                             scale=1.0 / SF1,
                                             bias=f1b_pc[:, fo:fo + 1])
                    for mo in range(DC):
                        if mlp_fp8:
                            nc.tensor.matmul(
                                ps_f2[mo][:],
                                f2_sl[:, :, mo * P:(mo + 1) * P], mch[:],
                                start=(fo2 == 0), stop=(fo2 == FFC // 2 - 1),
                                perf_mode=DR)
                        else:
                            for par in range(2):
                                nc.tensor.matmul(
                                    ps_f2[mo][:],
                                    f2_sl[:, par, mo * P:(mo + 1) * P],
                                    mch[:, par, :],
                                    start=(fo2 == 0 and par == 0),
                                    stop=(fo2 == FFC // 2 - 1 and par == 1))
                for mo in range(DC):
                    nc.vector.scalar_tensor_tensor(
                        x2T[b][:, mo, :], ps_f2[mo][:], 1.0 / SF2,
                        F(xT_f2b[:, mo, :]), op0=ALU.mult, op1=ALU.add)

            def phaseF(b, ps_scope):
                outT = act.tile([P, DC, T], f32, tag="act", name=f"outT_{b}")
                ln_pass(x2T[b], outT, lnpg1, lnpg2, ps_scope)
                nc.sync.dma_start(
                    out_d.ap()[b].rearrange("(c p) t -> p c t", p=P), outT[:])

            # ================= emission =================
            with tc.tile_pool(name="psA", bufs=3, space="PSUM") as psA:
                for b in range(BPC):
                    phaseA(b, psA)
            with (
                tc.tile_pool(name="psAux", bufs=1, space="PSUM") as psAux,
                tc.tile_pool(name="psSC", bufs=2, space="PSUM") as psSC,
                tc.tile_pool(name="psCTX", bufs=2, space="PSUM") as psCTX,
            ):
                phaseB(0, (psAux, psSC, psCTX))
                phaseB(1, (psAux, psSC, psCTX))
            with (
                tc.tile_pool(name="psC", bufs=2, space="PSUM") as psC,
                tc.tile_pool(name="psST", bufs=1, space="PSUM") as psST,
                tc.tile_pool(name="psLB", bufs=2, space="PSUM") as psLB,
            ):
                phaseC(0, psC)
                phaseD(0, (psST, psLB))
                phaseC(1, psC)
                phaseD(1, (psST, psLB))
            with (
                tc.tile_pool(name="psF1", bufs=2, space="PSUM") as psF1,
                tc.tile_pool(name="psF2", bufs=6, space="PSUM") as psF2,
            ):
                phaseE(0, psF1, psF2)
                phaseE(1, psF1, psF2)
            with (
                tc.tile_pool(name="psST2", bufs=1, space="PSUM") as psST2,
                tc.tile_pool(name="psLB2", bufs=2, space="PSUM") as psLB2,
            ):
                phaseF(0, (psST2, psLB2))
                phaseF(1, (psST2, psLB2))

    nc.compile()
    return nc


def _get_nc(use_bv: bool, use_bo: bool, use_f2b: bool, mlp_fp8: bool):
    key = ("nc", use_bv, use_bo, use_f2b, mlp_fp8)
    if key not in _cached:
        _cached[key] = _build(use_bv, use_bo, use_f2b, mlp_fp8)
    return _cached[key]


def kernel(query, key, wq, bq, wk, bk, wv, bv, wo, bo,
           ln2_g, ln2_b, fc1_w, fc1_b, fc2_w, fc2_b, lnp_g, lnp_b):
    import ml_dtypes
    from concourse.bass_utils import run_bass_kernel_spmd

    f = np.float32
    e4 = ml_dtypes.float8_e4m3
    c = np.ascontiguousarray
    query = np.asarray(query, f)
    key = np.asarray(key, f)
    use_bv = bool(np.any(np.asarray(bv)))
    use_bo = bool(np.any(np.asarray(bo)))
    use_f2b = bool(np.any(np.asarray(fc2_b)))
    nc = _get_nc(use_bv, use_bo, use_f2b, MLP_FP8)

    def wtile(w, scale, n_out):
        # [D_in, D_out] -> [P, n_out, DC_in, P] e4m3
        a = (np.asarray(w, f).T * np.float32(scale))
        d_in = a.shape[0]
        return c(a.reshape(d_in // P, P, n_out, P).transpose(1, 2, 0, 3)
                 .astype(e4))

    def pcol(v, scale=1.0):
        return c((np.asarray(v, f) * np.float32(scale))
                 .reshape(-1, P).T.astype(f))

    mdt = e4 if MLP_FP8 else ml_dtypes.bfloat16
    msc = np.float32(SF1 if MLP_FP8 else 1.0)
    msc2 = np.float32(SF2 if MLP_FP8 else 1.0)

    def g3(g, b):
        a = np.zeros((3, DC, P), f)
        a[0] = np.asarray(g, f).reshape(DC, P)
        a[1] = a[0]
        a[2] = -np.asarray(b, f).reshape(DC, P)
        return c(a)

    shared = {
        "wq8": wtile(wq, SCALE * SQ, DC),
        "wk8": wtile(wk, SK, DC),
        "wv8": c((np.asarray(wv, f).T * np.float32(SV))
                 .reshape(DC, P, D).transpose(1, 0, 2).astype(e4)),
        "wo8": wtile(wo, SO, DC),
        "fc18": c((np.asarray(fc1_w, f).T * msc)
                  .reshape(DC, P, FFC, P).transpose(1, 2, 0, 3).astype(mdt)),
        "fc28": c((np.asarray(fc2_w, f).T * msc2)
                  .reshape(FFC, P, D).transpose(1, 0, 2).astype(mdt)),
        "bqv": pcol(bq, SCALE * SQ),
        "bkv": pcol(bk, SK),
        "bvv": c(np.asarray(bv, f).reshape(1, D)),
        "bov": pcol(bo),
        "f1b": pcol(fc1_b),
        "f2b": pcol(fc2_b),
        "ln2g3": g3(ln2_g, ln2_b),
        "lnpg3": g3(lnp_g, lnp_b),
        "ones_col": np.ones((P, 1), f),
        "ones_row": np.ones((1, T), f),
        "sel8v": np.full((1, P), SCTX, f),
    }
    in_maps = []
    for core in range(NCORES):
        sl = slice(core * BPC, (core + 1) * BPC)
        m = dict(shared)
        qs = query[sl].transpose(0, 2, 1)
        ks = key[sl].transpose(0, 2, 1)
        m["qT"] = c(qs)
        m["qT8"] = c(qs.astype(e4))
        m["kT8"] = c(ks.astype(e4))
        in_maps.append(m)

    res = run_bass_kernel_spmd(nc, in_maps, core_ids=list(range(NCORES)))
    kernel._last_result = res
    out = np.concatenate([r["out"] for r in res.results], axis=0)
    return c(out.transpose(0, 2, 1))
